# revision 57
# baseline (speedup 1.0000x reference)
"""Trainium2 Bass kernel for BinaryMLP:
    h = relu(x @ sign(w1).T + b1); h = relu(h @ sign(w2).T + b2);
    h = relu(h @ sign(w3).T + b3); y = h @ w4.T + b4

Data-parallel over 8 NeuronCores: batch 65536 -> 8192 rows/core, weights
replicated. On-device dataflow is feature-major ("transposed"): activations
live in SBUF as [feature_partition, batch_free] so every layer's contraction
dim (the feature/hidden dim) is the PE partition dim. The host only slices
the batch, transposes/casts/zero-pads for layout, and concatenates the
result back.

Compute is bf16 on the tensor engine (binary +-1 weights are exact in bf16;
PSUM accumulates fp32). Structural PE savings vs the plain 7x112 / 4-pass
mapping (matmul time is moving-cols regardless of contraction rows, so
partial-row passes waste the array):

* fc1 runs 6 full 128-row k-passes + ONE row-tiled tail pass: the 784 = 6*128
  + 16 remainder is host-replicated 4x at partition offsets 0/32/64/96
  (x and w1 padded to 896 rows), and the 4 m-tiles' 16-row tail matmuls run
  CONCURRENTLY in 4x-row-tiling mode (tile_position=(32m, 0)), each
  accumulating into its own PSUM bank. 28 -> ~25.3 pass-times. Entering the
  (32,128) mode costs a full pipeline drain (~300ns, measured), so within a
  steady pair chunk B's tails ride in chunk A's tail window: they OPEN B's
  banks (start=True) and B's mains close them — one drain per pair, and A's
  relus fall right after A's tails so they drain under B's 24 passes (fc2-A
  never waits on relu latency).
* fc4 (512->10) runs 4 col-tiled k-passes CONCURRENTLY in (128,32) mode
  (tile_position=(0, 32j), partials at PSUM partitions 32j, padded to 32
  with zero weight columns so the bank is fully written). The
  cross-partition reduce of the 4 partials is a [128x10] 0/1 selector
  matmul in the same (128,32) mode, DEFERRED BY ONE PAIR: the scalar
  Identity evacuation of the partial bank to SBUF bf16 gets a full pair
  (~25us) of slack, so the reduce matmul enters the PE with its input
  long ready instead of stalling on the ACT queue. fc4 PE time: 4 ->
  ~2.5 pass-times per chunk. The last (half-width) pair uses the plain
  4-pass head so the end-of-kernel drain carries no copy round-trip.
  (A gpsimd accumulate-DMA reduce was tried instead and regressed badly:
  SWDGE descriptor generation costs ~1us of gpsimd time per DMA, and the
  4-DMA chain backed up pcp/PSUM recycling into a periodic ~2us PE
  stall.)

The binary-layer latents ship as fp8 e4m3 (halves the critical prologue
stream) and binarization reads ONLY the sign bit on the vector engine:
m = (w8 & 0x80) in-place, then bf16 out = m * (-1/64) + 1 -> exact +-1.
IEEE rounding preserves the sign even when tiny latents underflow to +-0,
so this matches where(w_fp32 >= 0, 1, -1) exactly.

Prologue is scheduled around the billed window (first framework memset ->
last NEFF instruction): w1 and x-chunk-0/1 stream on the single saturated
sync HWDGE ring in per-k-slice descriptors ordered by PE consumption, biases
and the tiny fc4 constants ride the gpsimd SWDGE ring. The HAM clock gate
needs ~3-5us of SUSTAINED full-array PE power to step 4/8 -> 8/8 (2.4 GHz),
and any PE idle gap resets the ramp - so warm-up matmuls drive the full
128x128 array and filler dummies are woven into the DMA-paced fc1 stalls.
Chunks are processed in interleaved pairs (kills layer-boundary stalls; peak
PSUM use stays at 8 banks). First and last pairs run at half width (256
cols): the lead pair lets the PE start on a quarter of the data; the tail
pair halves the end-of-kernel drain that is serial after the last matmul.
"""

import numpy as np

N_CORES = 8
F_IN = 784  # input features: 6 k-tiles of 128 + 16-row tail
K1 = 128
NK1 = 6  # full k-slices; slice index 6 is the replicated 16-row tail
TAILK = 16
F_PAD = 896  # 7 * 128: rows 768+32m..768+32m+16 hold the tail replica
H = 512  # hidden width: 4 k-tiles / m-tiles of 128
NKH = 4
N_OUT = 10
CHUNK = 512  # batch columns per moving-operand chunk
WARMUP = 12  # N=128 full-array PE warm-up matmuls bridging to the first real
# fc1 (measured: real fc1 work is available ~1us after the first filler; a
# longer warm-up just delays it — the HAM clock ramp runs on wall-clock
# sustained activity, which the real DMA-paced fc1 stream provides)


def chunk_schedule(b_shard: int, chunk: int = CHUNK):
    """Lead pair and tail pair at half width, full-width chunks between."""
    lead = [chunk // 2, chunk // 2]
    tail = [chunk // 2, chunk // 2]
    assert (b_shard - sum(lead) - sum(tail)) % chunk == 0
    widths = lead + [chunk] * ((b_shard - sum(lead) - sum(tail)) // chunk) + tail
    chunks = []
    off = 0
    for cw in widths:
        chunks.append((off, cw))
        off += cw
    return chunks


def build_nc(b_shard: int, num_devices: int = N_CORES, chunk: int = CHUNK):
    """Build + compile the per-core Bass program for a batch shard of
    b_shard columns. Every core runs the identical program."""
    import concourse.bacc as bacc
    import concourse.mybir as mybir
    import concourse.tile as tile

    f32 = mybir.dt.float32
    bf16 = mybir.dt.bfloat16
    ActFn = mybir.ActivationFunctionType
    AluOp = mybir.AluOpType

    # chunk schedule: lead pair and tail pair at half width (fast PE start /
    # short drain), full-width chunks in between. 256 is the floor: narrower
    # matmuls can no longer hide the ~97ns LDWEIGHTS behind the moving pass.
    chunks = chunk_schedule(b_shard, chunk)

    nc = bacc.Bacc(
        "TRN2", target_bir_lowering=False, debug=False, num_devices=num_devices
    )

    # NOTE on layouts: big streams ship in the (a p) n feature-major layout
    # (slice-k rows contiguous in HBM, 1KB lines). A host-packed
    # per-partition-contiguous layout (7KB lines) was tried and LIFTED DMA
    # rate 235->400KB/us but slowed EVERY concurrent matmul ~215->265ns —
    # long per-partition SBUF write bursts stall the PE's 128-partition
    # moving-operand reads. Fine-grained partition-interleaved writes don't.
    xT = nc.dram_tensor("xT", [F_PAD, b_shard], bf16, kind="ExternalInput")
    # binary-layer latents ship as fp8 e4m3: only their SIGN BIT is read on
    # device ((w & 0x80) — exact even when tiny values underflow to +-0,
    # since IEEE rounding preserves the sign), and fp8 halves the critical
    # prologue stream (w1 is ahead of x0/x1 on the saturated sync ring)
    f8 = mybir.dt.float8e4
    u8 = mybir.dt.uint8
    w1T = nc.dram_tensor("w1T", [F_PAD, H], f8, kind="ExternalInput")
    w2T = nc.dram_tensor("w2T", [H, H], f8, kind="ExternalInput")
    w3T = nc.dram_tensor("w3T", [H, H], f8, kind="ExternalInput")
    # fc4 stationary, host-packed: col 32j+c = w4[c, 128j:128j+128] for c<10,
    # zero for 10<=c<32 (so the col-tiled partial fully writes its bank)
    w4P = nc.dram_tensor("w4P", [128, 128], bf16, kind="ExternalInput")
    # cross-partition reduce selector: sel[32j+c, c] = 1
    sel = nc.dram_tensor("sel", [128, N_OUT], bf16, kind="ExternalInput")
    # biases host-packed to per-partition layout: col 4*l+m = b{l+1}[m*128:(m+1)*128]
    ball = nc.dram_tensor("ball", [128, 12], f32, kind="ExternalInput")
    # b4 replicated at partition offsets 0 and 32: the two deferred reduce
    # matmuls are col-paired at tile_position (0,0)/(0,32), so chunk B's
    # bias ACT reads lanes 32:42 (engines are lane-locked)
    b4 = nc.dram_tensor("b4", [48, 1], f32, kind="ExternalInput")
    y = nc.dram_tensor("y", [N_OUT, b_shard], f32, kind="ExternalOutput")

    with tile.TileContext(nc) as tc:
        with (
            tc.tile_pool(name="wconst", bufs=1) as wpool,
            tc.tile_pool(name="wstage", bufs=2) as wstage,
            tc.tile_pool(name="xbf", bufs=4) as xbf_pool,
            tc.tile_pool(name="hbuf", bufs=12) as h_pool,
            tc.tile_pool(name="pcp", bufs=6) as pcp_pool,
            tc.tile_pool(name="yout", bufs=6) as y_pool,
            tc.tile_pool(name="psum", bufs=8, space="PSUM") as ps_pool,
        ):
            # PE warm-up seed first, on gpsimd (its queue is empty, so this
            # runs immediately after the framework memsets): the sooner the
            # warm-up starts, the sooner the HAM clock gate ramps to 8/8
            # (2.4 GHz) — which also lifts the DMA clocks. The gate ramps on
            # sustained PE POWER, so the seed must drive the full 128x128
            # array — a tiny stationary tile never triggers it.
            pe_seed = wpool.tile([128, 128], bf16, tag="pe_seed", name="pe_seed")
            nc.vector.memset(pe_seed[:], 1.0)
            # Sign bias: maps w==0 -> +1, matching where(w>=0,1,-1)
            sign_eps = wpool.tile([128, 1], f32, tag="sign_eps", name="sign_eps")
            nc.vector.memset(sign_eps[:], 1e-20)
            # dummy activation: pull the ACT table load off the critical path
            # (Relu — the table the fc1 relus actually need; Sign is unused
            # now that binarization reads the fp8 sign bit)
            warm = wpool.tile([1, 1], bf16, tag="warm", name="warm")
            nc.scalar.activation(warm[:], sign_eps[0:1, :], ActFn.Relu, bias=0.0)

            # tiny constant loads on the gpsimd SWDGE ring: keeps both HWDGE
            # rings and the scalar engine free for the critical-path w1/x0
            # stream and sign work (none are needed until fc1's relu / fc4)
            ballt = wpool.tile([128, 12], f32, tag="ballt", name="ballt")
            nc.gpsimd.dma_start(ballt[:], ball.ap()[:])
            b4t = wpool.tile([48, 1], f32, tag="b4t", name="b4t")
            nc.gpsimd.dma_start(b4t[:], b4.ap()[:])
            w4c = wpool.tile([128, 128], bf16, tag="w4c", name="w4c")
            nc.gpsimd.dma_start(w4c[:], w4P.ap()[:])
            selt = wpool.tile([128, N_OUT], bf16, tag="selt", name="selt")
            nc.gpsimd.dma_start(selt[:], sel.ap()[:])

            # PE warm-up: full-array dummy matmuls while the prologue DMAs
            # stream. These both bridge the DMA wait AND supply the sustained
            # high-power activity the HAM gate needs (~3us) to step the PE
            # clock 4/8 -> 8/8; filler() dummies are also woven into the
            # DMA-paced fc1 stalls below so the power signal never drops.
            pe_sink = ps_pool.tile([128, 512], f32, tag="ps", name="pe_sink")

            def filler(n, sink=None):
                for _ in range(n):
                    nc.tensor.matmul(
                        (sink if sink is not None else pe_sink)[:, 0:128],
                        lhsT=pe_seed[:], rhs=pe_seed[:],
                        start=True, stop=True,
                    )

            filler(WARMUP)

            # ---- prologue streams ----
            # sync ring: w1 / x0 k-slices interleaved so the first fc1
            # matmul's deps (w1 k0 + x0 k0) land after ~190KB of traffic.
            cwA = chunks[0][1]
            w1f = wstage.tile([K1, NK1 + 1, H], f8, tag="wstage", name="w1bf")
            xb0 = xbf_pool.tile([K1, NK1 + 1, cwA], bf16, tag="xb", name="xb0")
            w1src = w1T.ap().rearrange("(a p) n -> p a n", p=K1)
            x0src = xT.ap()[:, 0:cwA].rearrange("(a p) n -> p a n", p=K1)
            for wk, xk in (((0, 1), (0, 2)), ((1, 4), (2, 5)), ((4, 7), (5, 7))):
                nc.sync.dma_start(w1f[:, wk[0]:wk[1], :], w1src[:, wk[0]:wk[1], :])
                nc.sync.dma_start(xb0[:, xk[0]:xk[1], :], x0src[:, xk[0]:xk[1], :])

            # x1 rides the same sync ring AFTER the pair-0 critical stream:
            # the ring is HBM-saturated during the prologue, so packet order
            # must match PE consumption order (a second ring just preempts
            # the critical packets, it doesn't add bandwidth)
            coff1, cw1 = chunks[1]
            xb1 = xbf_pool.tile([K1, NK1 + 1, cw1], bf16, tag="xb", name="xb1")
            nc.sync.dma_start(
                xb1[:], xT.ap()[:, coff1:coff1 + cw1].rearrange("(a p) n -> p a n", p=K1)
            )

            # ---- binarization from the fp8 sign bit ----
            # op1 (DVE, in-place): m = w8 & 0x80 -> {0, 128} as u8
            # op2 (per engs char): out_bf16 = m * (-1/64) + 1 -> {+1, -1};
            #   "s" routes op2 to the scalar engine (activation Identity with
            #   scale/bias), balancing the two engines like the old split.
            # +-0 keeps its sign bit under IEEE rounding, so this matches
            # where(w>=0,1,-1) exactly for every latent that rounds to zero.
            def sign_slice(wb, wf, k, k_size, eng, name):
                nc.vector.tensor_scalar(
                    wf[:, k, :].bitcast(u8), wf[:, k, :].bitcast(u8),
                    0x80, None, AluOp.bitwise_and,
                )
                if eng == "s":
                    nc.scalar.activation(
                        wb[:, k, :], wf[:, k, :].bitcast(u8), ActFn.Identity,
                        bias=1.0, scale=-0.015625,
                    )
                else:
                    nc.vector.tensor_scalar(
                        wb[:, k, :], wf[:, k, :].bitcast(u8),
                        -0.015625, 1.0,
                        AluOp.mult, AluOp.add,
                    )

            # engine split: k0 on scalar (idle at that point), the rest on
            # vector — the single-op bitwise sign is ~2.5x cheaper than the
            # scalar Sign activation, and the scalar engine is the busier
            # one later (relu m0/m1 + head activations)
            # all-vector: an "s" slice would queue op2 behind the warm act's
            # ~1.3us table load on the scalar FIFO, gating the first real MM
            w1b = wpool.tile([K1, NK1 + 1, H], bf16, tag="w1b", name="w1b")
            for k, eng in enumerate("vvvvvvv"[:NK1 + 1]):
                sign_slice(w1b, w1f, k, K1, eng, "w1")

            def load_x(ci, after=None, tail_first=False):
                coff, cw = chunks[ci]
                xb = xbf_pool.tile([K1, NK1 + 1, cw], bf16, tag="xb", name=f"xb{ci}")
                src = xT.ap()[:, coff:coff + cw].rearrange("(a p) n -> p a n", p=K1)
                if tail_first:
                    # chunk B's FIRST consumer is its tail group (it opens
                    # B's banks inside A's tail window), and the tail plane
                    # is slice 6 — the LAST thing a single in-order DMA
                    # writes. Ship slice 6 ahead of slices 0-5 so the tails
                    # never wait on the 918KB stream (measured ~0.5us/pair).
                    dmas = [
                        nc.sync.dma_start(xb[:, NK1, :], src[:, NK1, :]),
                        nc.sync.dma_start(xb[:, 0:NK1, :], src[:, 0:NK1, :]),
                    ]
                else:
                    dmas = [nc.sync.dma_start(xb[:], src)]
                if after is not None:
                    for dma in dmas:
                        tile.add_dep_helper(dma.ins, after.ins, sync=True)
                return xb

            def prep_bin(w_dram, n_k, k_size, name, engs, after=None, splits=1,
                         rings=None):
                # rings: per-split issuing engine; the scalar engine owns the
                # second HWDGE ring (qActDynamicHW), so a split issued there
                # streams concurrently with the sync ring's x traffic
                wf = wstage.tile([k_size, n_k, H], f8, tag="wstage", name=f"{name}f")
                src = w_dram.ap().rearrange("(a p) n -> p a n", p=k_size)
                bounds = [round(n_k * s / splits) for s in range(splits + 1)]
                for s in range(splits):
                    k0, k1 = bounds[s], bounds[s + 1]
                    eng = (rings[s] if rings else nc.sync)
                    dma = eng.dma_start(wf[:, k0:k1, :], src[:, k0:k1, :])
                    if after is not None:
                        tile.add_dep_helper(dma.ins, after.ins, sync=True)
                wb = wpool.tile([k_size, n_k, H], bf16, tag=name, name=name)
                for k in range(n_k):
                    sign_slice(wb, wf, k, k_size, engs[k], name)
                return wb

            b1t = ballt[:, 0:4]
            b2t = ballt[:, 4:8]
            b3t = ballt[:, 8:12]

            def relu_bank(c, cw, pss, m, btiles, name):
                ht = h_pool.tile(
                    [128, cw], bf16, tag=f"h{name}", name=f"h{name}_{c}_{m}"
                )
                if m < 2:
                    nc.scalar.activation(
                        ht[:], pss[m][:], ActFn.Relu,
                        bias=btiles[:, m : m + 1], scale=1.0,
                    )
                else:
                    # same math on the otherwise-idle vector engine:
                    # out = max(in + bias, 0) -> halves the relu drain
                    # latency that gates PSUM-bank recycling
                    nc.vector.tensor_scalar(
                        ht[:], pss[m][:], btiles[:, m : m + 1], 0.0,
                        AluOp.add, AluOp.max,
                    )
                return ht

            def fc1_alloc(c, cw):
                return [
                    ps_pool.tile([128, cw], f32, tag="ps", name=f"ps_1_{c}_{m}")
                    for m in range(NKH)
                ]

            def fc1_mains(c, cw, xb, pss, k_outer=False, fill=0, first=True,
                          close=False):
                """24 full 128-row fc1 matmuls for one chunk. first/close pick
                which end of the accumulation group the row-tiled tail is on
                (PSUM has_written accumulation is order-free; only the first
                writer must clear)."""
                mms = []
                order = (
                    [(m, k) for k in range(NK1) for m in range(NKH)]
                    if k_outer
                    else [(m, k) for m in range(NKH) for k in range(NK1)]
                )
                for m, k in order:
                    mms.append(
                        nc.tensor.matmul(
                            pss[m][:],
                            lhsT=w1b[:, k, m * 128 : (m + 1) * 128],
                            rhs=xb[:, k, :],
                            start=(first and k == 0),
                            stop=(close and k == NK1 - 1),
                        )
                    )
                    if fill and m == NKH - 1 and k < NK1 - 1:
                        filler(fill)
                return mms

            def fc1_tails(c, xb, pss, start, stop):
                """4 concurrent 16-row tail matmuls (4x row tiling). Tail
                stationary/moving replicas sit at partition offsets 32m
                (host-packed plane 6 of w1T/xT)."""
                for m in range(NKH):
                    nc.tensor.matmul(
                        pss[m][:],
                        lhsT=w1b[32 * m : 32 * m + TAILK, NK1, m * 128 : (m + 1) * 128],
                        rhs=xb[32 * m : 32 * m + TAILK, NK1, :],
                        start=start,
                        stop=stop,
                        tile_position=(32 * m, 0),
                    )

            def fc1_relus(c, cw, pss):
                return [relu_bank(c, cw, pss, m, b1t, "1") for m in range(NKH)]

            def layer(c, cw, ins_of_k, wtiles, btiles, n_k, name):
                mms = []
                pss = [
                    ps_pool.tile([128, cw], f32, tag="ps", name=f"ps_{name}_{c}_{m}")
                    for m in range(NKH)
                ]
                for m in range(NKH):
                    for k in range(n_k):
                        mms.append(
                            nc.tensor.matmul(
                                pss[m][:],
                                lhsT=wtiles[:, k, m * 128 : (m + 1) * 128],
                                rhs=ins_of_k(k),
                                start=(k == 0),
                                stop=(k == n_k - 1),
                            )
                        )
                outs = [relu_bank(c, cw, pss, m, btiles, name) for m in range(NKH)]
                return mms, outs

            def head_partials(c, cw, h3):
                """fc4: 4 col-tiled k-passes run concurrently in (128,32)
                mode; partial j lands at PSUM partitions 32j (32 rows fully
                written - weight cols 10:32 are zero)."""
                ps4 = ps_pool.tile([128, cw], f32, tag="ps", name=f"ps4_{c}")
                for j in range(NKH):
                    nc.tensor.matmul(
                        ps4[32 * j : 32 * j + 32, :],
                        lhsT=w4c[:, 32 * j : 32 * j + 32],
                        rhs=h3[j][:],
                        start=True,
                        stop=True,
                        tile_position=(0, 32 * j),
                    )
                return ps4

            def head_evac(c, cw, ps4):
                """ACT evacuation of the fc4 partial bank to SBUF bf16; the
                selector matmul consumes it one pair later."""
                pcp = pcp_pool.tile([128, cw], bf16, tag="pcp", name=f"pcp_{c}")
                nc.scalar.activation(
                    pcp[:], ps4[:], ActFn.Identity, bias=0.0, scale=1.0
                )
                return pcp

            def head_mmreduce2(pending):
                """Selector matmuls collapse the 32j partition groups of the
                evacuated partials, then bias + store. The two chunks'
                reduces are col-paired at tile_position (0,0)/(0,32) into one
                PSUM bank so they run concurrently on the PE."""
                if not pending:
                    return
                cw_max = max(chunks[ci][1] for ci, _ in pending)
                ps5 = ps_pool.tile(
                    [48, cw_max], f32, tag="ps", name=f"ps5_{pending[0][0]}"
                )
                for i, (ci, pcp) in enumerate(pending):
                    cw = chunks[ci][1]
                    nc.tensor.matmul(
                        ps5[32 * i : 32 * i + N_OUT, 0:cw],
                        lhsT=selt[:],
                        rhs=pcp[:],
                        start=True,
                        stop=True,
                        tile_position=(0, 32 * i),
                    )
                for i, (ci, pcp) in enumerate(pending):
                    coff, cw = chunks[ci]
                    yt = y_pool.tile([48, cw], f32, tag="yt", name=f"yt_{ci}")
                    nc.scalar.activation(
                        yt[32 * i : 32 * i + N_OUT, :],
                        ps5[32 * i : 32 * i + N_OUT, 0:cw],
                        ActFn.Identity,
                        bias=b4t[32 * i : 32 * i + N_OUT, :],
                        scale=1.0,
                    )
                    nc.sync.dma_start(
                        y.ap()[:, coff : coff + cw],
                        yt[32 * i : 32 * i + N_OUT, :],
                    )

            def head_plain(c, coff, cw, h3):
                """Baseline 4-pass fc4 for the final pair: short serial chain
                after the last matmul (no SWDGE round-trip on the drain)."""
                ps4 = ps_pool.tile([N_OUT, cw], f32, tag="ps", name=f"ps4p_{c}")
                for k in range(NKH):
                    nc.tensor.matmul(
                        ps4[:],
                        lhsT=w4c[:, 32 * k : 32 * k + N_OUT],
                        rhs=h3[k][:],
                        start=(k == 0),
                        stop=(k == NKH - 1),
                    )
                yt = y_pool.tile([N_OUT, cw], f32, tag="yt", name=f"yt_{c}")
                nc.scalar.activation(
                    yt[:], ps4[:], ActFn.Identity, bias=b4t[0:N_OUT, :], scale=1.0
                )
                nc.sync.dma_start(y.ap()[:, coff : coff + cw], yt[:])

            # ---- pair 0: weight prep interleaved with the layer flow so
            # later loads' packets queue behind what's needed first ----
            cwB = chunks[1][1]
            # pair 0 keeps tails AFTER each chunk's mains: the tail planes
            # land last in the DMA stream, and a stalled tail group at the
            # queue head would block everything behind it
            pssA = fc1_alloc(0, cwA)
            mmsA = fc1_mains(0, cwA, xb0, pssA, k_outer=True, fill=1)
            fc1_tails(0, xb0, pssA, start=False, stop=True)
            h1A = fc1_relus(0, cwA, pssA)
            pssB = fc1_alloc(1, cwB)
            mmsB = fc1_mains(1, cwB, xb1, pssB)
            fc1_tails(1, xb1, pssB, start=False, stop=True)
            h1B = fc1_relus(1, cwB, pssB)

            # NOTE: both HWDGE rings share the 16 SDMA engines round-robin at
            # packet granularity, so routing w2 via the scalar ring does NOT
            # add bandwidth — it just preempts the critical w1/x0/x1 stream
            # (measured: a 4.2us fc2-A stall). Keep everything on sync, in
            # consumption order.
            w2b = prep_bin(w2T, NKH, 128, "w2b", "vvvv", after=mmsA[0], splits=2)
            # fc1-B's 4th psum allocation above evicted pe_sink's bank; a new
            # sink bridges the fc1-B -> fc2-A gap while w2 streams + signs
            pe_sink2 = ps_pool.tile([128, 512], f32, tag="ps", name="pe_sink2")
            filler(2, sink=pe_sink2)
            _, h2A = layer(0, cwA, lambda k: h1A[k][:], w2b, b2t, NKH, "2")
            _, h2B = layer(1, cwB, lambda k: h1B[k][:], w2b, b2t, NKH, "2")

            w3b = prep_bin(w3T, NKH, 128, "w3b", "vvvv", after=mmsB[0])
            _, h3A = layer(0, cwA, lambda k: h2A[k][:], w3b, b3t, NKH, "3")
            _, h3B = layer(1, cwB, lambda k: h2B[k][:], w3b, b3t, NKH, "3")

            ps4A = head_partials(0, cwA, h3A)
            ps4B = head_partials(1, cwB, h3B)
            pending = [(0, head_evac(0, cwA, ps4A)), (1, head_evac(1, cwB, ps4B))]

            # ---- remaining pairs ----
            def do_pair(pair, xbs, next_pair, pending, last=False):
                # finish (tails + relus) chunk A BEFORE chunk B's mains: the
                # relus then drain under B's 24 passes, so fc2-A never waits
                # on the relu latency (closing all 8 banks at once was a
                # measured 1.5us/pair stall). Chunk B's tails ride in the
                # SAME (32,128) window as A's — they OPEN B's banks
                # (start=True) and B's mains close them — so the pair pays
                # one tail-window mode drain instead of two.
                cA, cB = pair
                cwA_, cwB_ = chunks[cA][1], chunks[cB][1]
                pssA_ = fc1_alloc(cA, cwA_)
                fc1_mains(cA, cwA_, xbs[0], pssA_)
                fc1_tails(cA, xbs[0], pssA_, start=False, stop=True)
                pssB_ = fc1_alloc(cB, cwB_)
                fc1_tails(cB, xbs[1], pssB_, start=True, stop=False)
                h1A_ = fc1_relus(cA, cwA_, pssA_)
                fc1_mains(cB, cwB_, xbs[1], pssB_, first=False, close=True)
                hs = [h1A_, fc1_relus(cB, cwB_, pssB_)]
                for name, wb, bt in (("2", w2b, b2t), ("3", w3b, b3t)):
                    hs = [
                        layer(
                            ci, chunks[ci][1], lambda k, h=hs[i]: h[k][:], wb, bt,
                            NKH, name,
                        )[1]
                        for i, ci in enumerate(pair)
                    ]
                # previous pair's deferred reduces enter the (128,32) window
                # first: their evacuated inputs have been ready for a full
                # pair, so the PE never waits on the ACT copy
                #
                # PSUM ring-phase padding: a steady pair makes 27 psum
                # allocations against the 8-slot ring, so the slot phase
                # drifts 3/pair and ps5/ps4A/ps4B banks (freed LATE, by ACT
                # evacuations) get reused by next-pair fc1 banks whose first
                # write comes early — measured ~0.5-1us/pair of B-tail sem
                # waits. Dead pad allocations (no instructions) bring the
                # count to 32: phase is stable and the late-freed banks map
                # onto pssA[1]/pssB[0]/pssB[1], all of which first write
                # >=2us after the evacuations complete.
                def pad(n):
                    for j in range(n):
                        ps_pool.tile([1, 1], f32, tag="ps",
                                     name=f"pspad_{pair[0]}_{j}")

                # prefetch the NEXT pair's x loads BEFORE the y stores:
                # a y-store dma_start whose yt isn't ready head-blocks the
                # strict sync FIFO for ~most of a pair (measured wait=4943 on
                # the blocked descriptor), starving the next pair's x stream
                # and stalling its tail matmuls on the DMA semaphore
                xbs_next = (
                    [
                        load_x(ci, tail_first=(i == 1))
                        for i, ci in enumerate(next_pair)
                    ]
                    if next_pair is not None
                    else None
                )
                pad(1)
                head_mmreduce2(pending)
                if last:
                    for i, ci in enumerate(pair):
                        head_plain(ci, chunks[ci][0], chunks[ci][1], hs[i])
                    return [], xbs_next
                pad(2)
                ps4s = [
                    head_partials(ci, chunks[ci][1], hs[i])
                    for i, ci in enumerate(pair)
                ]
                pcs = [
                    (ci, head_evac(ci, chunks[ci][1], ps4s[i]))
                    for i, ci in enumerate(pair)
                ]
                pad(2)
                return pcs, xbs_next

            pairs = [
                list(range(s, min(s + 2, len(chunks))))
                for s in range(2, len(chunks), 2)
            ]
            xbs_cur = [
                load_x(ci, after=mmsA[0], tail_first=(i == 1))
                for i, ci in enumerate(pairs[0])
            ]
            for pi, pair in enumerate(pairs):
                next_pair = pairs[pi + 1] if pi + 1 < len(pairs) else None
                pending, xbs_cur = do_pair(
                    pair,
                    xbs_cur,
                    next_pair,
                    pending,
                    last=(pi == len(pairs) - 1),
                )

    nc.compile()
    return nc


_CACHE = {}


def _get_nc(b_shard: int):
    key = b_shard
    if key not in _CACHE:
        _CACHE[key] = build_nc(b_shard)
    return _CACHE[key]


def make_in_maps(x, w1, b1, w2, b2, w3, b3, w4, b4, n_cores=N_CORES):
    """Host-side layout prep (slicing/transpose/dtype/zero-pad marshalling)."""
    import ml_dtypes

    B = x.shape[0]
    b_shard = B // n_cores

    def pad_tail(aT):
        # [784, n] -> [896, n]: rows 768+32m..+16 = rows 768:784, rest zero
        out = np.zeros((F_PAD, aT.shape[1]), dtype=aT.dtype)
        out[:768] = aT[:768]
        for m in range(4):
            out[768 + 32 * m : 768 + 32 * m + TAILK] = aT[768:784]
        return out

    xT = pad_tail(
        np.ascontiguousarray(np.asarray(x, dtype=np.float32).T).astype(
            ml_dtypes.bfloat16
        )
    )
    ball = np.concatenate(
        [np.asarray(b, np.float32).reshape(NKH, 128).T for b in (b1, b2, b3)], axis=1
    )

    def wprep(w):
        return np.ascontiguousarray(
            np.asarray(w, np.float32).T.astype(ml_dtypes.bfloat16)
        )


    w4T = wprep(w4)  # [512, 10]
    w4P = np.zeros((128, 128), dtype=ml_dtypes.bfloat16)
    for j in range(4):
        w4P[:, 32 * j : 32 * j + N_OUT] = w4T[128 * j : 128 * (j + 1), :]
    sel = np.zeros((128, N_OUT), dtype=ml_dtypes.bfloat16)
    for j in range(4):
        for c in range(N_OUT):
            sel[32 * j + c, c] = 1.0
    b4rep = np.zeros((48, 1), dtype=np.float32)
    b4rep[0:N_OUT, 0] = np.asarray(b4, np.float32)
    b4rep[32 : 32 + N_OUT, 0] = np.asarray(b4, np.float32)

    def wprep8(w):
        return np.ascontiguousarray(
            np.asarray(w, np.float32).T.astype(ml_dtypes.float8_e4m3)
        )

    common = {
        "w1T": np.ascontiguousarray(pad_tail(wprep8(w1))),
        "w2T": wprep8(w2),
        "w3T": wprep8(w3),
        "w4P": np.ascontiguousarray(w4P),
        "sel": np.ascontiguousarray(sel),
        "ball": np.ascontiguousarray(ball),
        "b4": np.ascontiguousarray(b4rep),
    }
    return [
        {"xT": np.ascontiguousarray(xT[:, i * b_shard : (i + 1) * b_shard]), **common}
        for i in range(n_cores)
    ]


def kernel(x, w1, b1, w2, b2, w3, b3, w4, b4):
    from concourse.bass_utils import run_bass_kernel_spmd

    B = x.shape[0]
    b_shard = B // N_CORES
    nc = _get_nc(b_shard)
    in_maps = make_in_maps(x, w1, b1, w2, b2, w3, b3, w4, b4)
    res = run_bass_kernel_spmd(nc, in_maps, core_ids=list(range(N_CORES)))
    yT = np.concatenate([res.results[i]["y"] for i in range(N_CORES)], axis=1)
    return np.ascontiguousarray(yT.T).astype(np.float32)
